# revision 1
# baseline (speedup 1.0000x reference)
"""Trainium2 Bass kernel for nn_NatureCNN (CNN trunk + MLP + 2x2-layer LSTM + FC head).

Sharding: data-parallel over the segment axis. Core k owns segments
b in [4k, 4k+4) of both LSTM stacks, i.e. frames [128k,128k+128) (rnns
block A) and [1024+128k, +128) (rnn block B) -- 256 frames/core, fully
independent per core (zero communication). Host stacks the per-core
[4, 130] outputs.

On-chip layouts (channels-on-partitions):
  obsT   [48=(ic,ry,rx), 256f, (by,bx)=256]  phase-decomposed 4x4 blocks, bf16
  conv1  K=48, PSUM-accumulate over (dy,dx) block shifts; 4 col-tiles by
         output-row group j (rows {j, j+2, .., j+10}) -> R [(j,oc1)=128, ...]
  conv2  K=128=(ky2,ic2) directly on R; accumulate kx2 -> C2 [64, (f,6,6)]
  conv3  K=64; accumulate (ky3,kx3); 2 col-tiles by py3-pair -> C3
  MLP    dataT [1536, f] bf16 lin1; fp32r lin2 -> doutT
  LSTM   layer-skewed steps; gates layer0 at PSUM rows 0:4, layer1 at 32:36
         (col-tiling); one big tanh per step (sigmoid(x)=(tanh(x/2)+1)/2);
         scalar_tensor_tensor cell update; PE-transpose of h each step.
         c stored as c/2 ("sC"), h stored as 2h (0.5 folded into weights).
  head   feature-major fc1/fc2, batch on the moving free dim.
"""

import numpy as np
import ml_dtypes

import concourse.bass as bass
import concourse.mybir as mybir
import concourse.tile as tile
from concourse.bass_utils import run_bass_kernel_spmd

dt = mybir.dt
AF = mybir.ActivationFunctionType
ALU = mybir.AluOpType

N_CORES = 8
S, H = 32, 100
B_LOC = 4                          # segments per core
FPC = 2 * B_LOC * S                # 256 frames per core
FB = 32                            # frames per conv block
NBLK = FPC // FB
G4 = 4 * H                         # 400
DATA_DIM, RIN, OUT = 1500, 1536, 130
NSTEP = 2 * S + 1                  # 65 layer-skewed steps

bf = ml_dtypes.bfloat16

MAX_WAITS = 1
_ctr = [0]


def _fix_sync_waits(nc):
    """This env's walrus rejects >1 sem-wait per instruction; hoist extras
    onto same-engine NoOps inserted immediately before, preserving order."""
    for f in nc.m.functions:
        for bb in f.blocks:
            new_list = []
            for ins in bb.instructions:
                si = ins.sync_info
                if si is not None and si.on_wait and len(si.on_wait) > MAX_WAITS:
                    waits = list(si.on_wait)
                    for i in range(0, len(waits) - MAX_WAITS, MAX_WAITS):
                        _ctr[0] += 1
                        nop = mybir.InstNoOp(name=f"waitfix-{_ctr[0]}", ins=[], outs=[])
                        nop.engine = ins.engine
                        nop.sync_info = mybir.SyncInfo(
                            on_wait=waits[i : i + MAX_WAITS], on_update=[])
                        new_list.append(nop)
                        nc.register_instruction(nop, overwrite=True)
                    si.on_wait = waits[len(waits) - MAX_WAITS :]
                    ins.sync_info = si
                new_list.append(ins)
            bb.instructions = new_list


# --------------------------------------------------------------------------
# host-side weight/layout prep
# --------------------------------------------------------------------------

def _gate_perm():
    # torch gate order (i, f, g, o) -> ours (f, i, o, g)
    return np.concatenate([np.arange(100, 200), np.arange(0, 100),
                           np.arange(300, 400), np.arange(200, 300)])


def _feat_ch_order():
    """our featT row R -> original feat channel index."""
    orig = np.zeros(RIN, np.int64)
    for R in range(1024):
        c, rem = divmod(R, 128)
        g, oc3 = divmod(rem, 64)
        orig[R] = oc3 * 16 + (2 * g + c // 4) * 4 + (c % 4)
    orig[1024:] = np.arange(1024, RIN)
    return orig


def _prep_weights(inp):
    w = {}
    gp = _gate_perm()
    ch = _feat_ch_order()

    c1 = inp['conv1_w'].reshape(32, 3, 2, 4, 2, 4)   # oc, ic, dy, ry, dx, rx
    w['w1'] = np.ascontiguousarray(
        c1.transpose(2, 4, 1, 3, 5, 0).reshape(2, 2, 48, 32)).astype(bf)
    w['cb1'] = np.tile(inp['conv1_b'], 4).reshape(128, 1).astype(np.float32)
    w['w2'] = np.ascontiguousarray(
        inp['conv2_w'].transpose(3, 2, 1, 0).reshape(4, 128, 64)).astype(bf)
    w['cb2'] = inp['conv2_b'].reshape(64, 1).astype(np.float32)
    w['w3'] = np.ascontiguousarray(
        inp['conv3_w'].transpose(2, 3, 1, 0).reshape(9, 64, 64)).astype(bf)
    w['cb3'] = np.tile(inp['conv3_b'], 2).reshape(128, 1).astype(np.float32)

    l1 = np.zeros((RIN, 1024), np.float32)
    l1[:DATA_DIM] = inp['lin1_w'].T
    w['lin1wT'] = l1.reshape(12, 128, 1024).astype(bf)
    w['b1col'] = inp['lin1_b'].reshape(8, 128).T.copy().astype(np.float32)
    w['lin2wT'] = inp['lin2_w'].T.reshape(8, 128, 512).copy().astype(np.float32)
    w['b2col'] = inp['lin2_b'].reshape(4, 128).T.copy().astype(np.float32)

    def lstm_prep(pfx):
        wih0 = inp[f'{pfx}_wih0'][gp].copy()
        wih0[300:400] *= 2.0
        wih0T = wih0.T[ch].copy()                     # [1536, 400]
        b0 = (inp[f'{pfx}_bih0'] + inp[f'{pfx}_bhh0'])[gp].copy()
        b0[300:400] *= 2.0
        whh0 = inp[f'{pfx}_whh0'][gp].copy()
        whh0[300:400] *= 2.0
        wih1 = inp[f'{pfx}_wih1'][gp].copy()
        wih1[300:400] *= 2.0
        whh1 = inp[f'{pfx}_whh1'][gp].copy()
        whh1[300:400] *= 2.0
        b1 = (inp[f'{pfx}_bih1'] + inp[f'{pfx}_bhh1'])[gp].copy()
        b1[300:400] *= 2.0
        whh1ext = np.concatenate([whh1.T * 0.5, b1[None, :]], 0)
        return wih0T, b0, whh0.T * 0.5, wih1.T * 0.5, whh1ext

    a = [lstm_prep('rnns'), lstm_prep('rnn')]
    w['wih0T'] = np.stack([x[0] for x in a]).reshape(2, 12, 128, G4).astype(bf)
    w['b0row'] = np.stack([x[1] for x in a]).reshape(2, 1, G4).astype(bf)
    w['whh0T'] = np.stack([x[2] for x in a]).astype(bf)
    w['wih1T'] = np.stack([x[3] for x in a]).astype(bf)
    w['whh1T'] = np.stack([x[4] for x in a]).astype(bf)

    w['fc1wT'] = np.stack([inp['fc1_w'][:, :100].T * 0.5,
                           inp['fc1_w'][:, 100:].T * 0.5]).astype(bf)
    w['fc1bcol'] = inp['fc1_b'].reshape(4, 128).T.copy().astype(np.float32)
    w['fc2wT'] = inp['fc2_w'].T.reshape(4, 128, OUT).copy().astype(bf)
    w['fc2brow'] = inp['fc2_b'].reshape(1, OUT).astype(bf)

    i4 = np.zeros((128, 4), np.float32)
    for k in range(4):
        i4[32 * k : 32 * k + 4] = np.eye(4)
    w['i4rep'] = i4.astype(bf)
    w['ident36'] = np.eye(36).astype(bf)
    w['ones128'] = np.ones((1, 128), np.float32).astype(bf)
    w['ones4'] = np.ones((1, 4), np.float32).astype(bf)
    w['hT_init'] = np.concatenate(
        [np.zeros((100, 36), np.float32), np.ones((1, 36), np.float32)]).astype(bf)
    w['zeros128'] = np.zeros((128, 512), np.float32)
    w['zerosbf'] = np.zeros((36, 100), np.float32).astype(bf)
    return w


def _prep_core_inputs(inp, w, k):
    idx = np.concatenate([np.arange(128 * k, 128 * k + 128),
                          np.arange(1024 + 128 * k, 1024 + 128 * k + 128)])
    obs = np.asarray(inp['observations'], np.float32)[idx]
    obsT = (obs.reshape(FPC, 3, 16, 4, 16, 4)
            .transpose(1, 3, 5, 0, 2, 4)
            .reshape(48, FPC, 256)).astype(bf)
    data = np.asarray(inp['data'], np.float32)[idx]
    dT = np.zeros((RIN, FPC), np.float32)
    dT[:DATA_DIM] = data.T
    m = {'obsT': obsT, 'dataT': np.ascontiguousarray(dT.reshape(12, 128, FPC)).astype(bf)}
    m.update(w)
    return m


# --------------------------------------------------------------------------
# kernel IR
# --------------------------------------------------------------------------

def _build_nc(debug=False):
    nc = bass.Bass("TRN2", target_bir_lowering=False, debug=False,
                   num_devices=N_CORES)

    D = {}
    def inp(name, shape, d):
        D[name] = nc.dram_tensor(name, shape, d, kind="ExternalInput")

    inp('obsT', [48, FPC, 256], dt.bfloat16)
    inp('dataT', [12, 128, FPC], dt.bfloat16)
    inp('w1', [2, 2, 48, 32], dt.bfloat16)
    inp('cb1', [128, 1], dt.float32)
    inp('w2', [4, 128, 64], dt.bfloat16)
    inp('cb2', [64, 1], dt.float32)
    inp('w3', [9, 64, 64], dt.bfloat16)
    inp('cb3', [128, 1], dt.float32)
    inp('lin1wT', [12, 128, 1024], dt.bfloat16)
    inp('b1col', [128, 8], dt.float32)
    inp('lin2wT', [8, 128, 512], dt.float32r)
    inp('b2col', [128, 4], dt.float32)
    inp('wih0T', [2, 12, 128, G4], dt.bfloat16)
    inp('b0row', [2, 1, G4], dt.bfloat16)
    inp('whh0T', [2, 100, G4], dt.bfloat16)
    inp('wih1T', [2, 100, G4], dt.bfloat16)
    inp('whh1T', [2, 101, G4], dt.bfloat16)
    inp('fc1wT', [2, 100, 512], dt.bfloat16)
    inp('fc1bcol', [128, 4], dt.float32)
    inp('fc2wT', [4, 128, OUT], dt.bfloat16)
    inp('fc2brow', [1, OUT], dt.bfloat16)
    inp('i4rep', [128, 4], dt.bfloat16)
    inp('ident36', [36, 36], dt.bfloat16)
    inp('ones128', [1, 128], dt.bfloat16)
    inp('ones4', [1, 4], dt.bfloat16)
    inp('hT_init', [101, 36], dt.bfloat16)
    inp('zeros128', [128, 512], dt.float32)
    inp('zerosbf', [36, 100], dt.bfloat16)

    out_d = nc.dram_tensor('out', [B_LOC, OUT], dt.float32, kind="ExternalOutput")
    xw_scr = nc.dram_tensor('xw_scr', [2, 128, G4], dt.bfloat16)  # internal scratch
    dbg = {}
    if debug:
        dbg['d_c3'] = nc.dram_tensor('d_c3', [128, FPC * 8], dt.bfloat16, kind="ExternalOutput")
        dbg['d_dout'] = nc.dram_tensor('d_dout', [128, 4 * FPC], dt.bfloat16, kind="ExternalOutput")
        dbg['d_xw'] = nc.dram_tensor('d_xw', [2, 128, 8 * G4], dt.bfloat16, kind="ExternalOutput")
        dbg['d_hT'] = nc.dram_tensor('d_hT', [101, 36], dt.bfloat16, kind="ExternalOutput")
        dbg['d_q'] = nc.dram_tensor('d_q', [100, 4], dt.bfloat16, kind="ExternalOutput")

    with tile.TileContext(nc) as tc:
        with (
            tc.tile_pool(name="const", bufs=1) as cpool,
            tc.tile_pool(name="acts", bufs=1) as apool,
            tc.tile_pool(name="conv", bufs=2) as vpool,
            tc.tile_pool(name="lstm", bufs=2) as lpool,
        ):
            # ---- resident constants ----
            def ld(name, shape, d, tag=None):
                t = cpool.tile(shape, d, tag=tag or name)
                nc.sync.dma_start(t[:], D[name][:])
                return t

            def ld_stack(name, p, a, wdt, inner):
                # DRAM [a, p, inner] -> SBUF [p, a*inner]
                t = cpool.tile([p, a * inner], wdt, tag=name)
                nc.sync.dma_start(t[:].rearrange("p (a g) -> p a g", a=a),
                                  D[name][:].rearrange("a p g -> p a g"))
                return t

            w1_s = cpool.tile([48, 4 * 32], dt.bfloat16, tag="w1")
            nc.sync.dma_start(w1_s[:].rearrange("p (a b) -> p a b", a=4),
                              D['w1'][:].rearrange("a c p o -> p (a c) o"))
            cb1_s = ld('cb1', [128, 1], dt.float32)
            w2_s = ld_stack('w2', 128, 4, dt.bfloat16, 64)
            cb2_s = ld('cb2', [64, 1], dt.float32)
            w3_s = ld_stack('w3', 64, 9, dt.bfloat16, 64)
            cb3_s = ld('cb3', [128, 1], dt.float32)
            b1c_s = ld('b1col', [128, 8], dt.float32)
            b2c_s = ld('b2col', [128, 4], dt.float32)
            i4_s = ld('i4rep', [128, 4], dt.bfloat16)
            id36_s = ld('ident36', [36, 36], dt.bfloat16)
            ones128_s = ld('ones128', [1, 128], dt.bfloat16)
            ones4_s = ld('ones4', [1, 4], dt.bfloat16)
            zeros_s = ld('zeros128', [128, 512], dt.float32)

            lin1w_s = cpool.tile([128, 12 * 1024], dt.bfloat16, tag="lin1w")
            nc.sync.dma_start(lin1w_s[:].rearrange("p (a g) -> p a g", a=12),
                              D['lin1wT'][:].rearrange("a p g -> p a g"))
            lin2w_s = cpool.tile([128, 8 * 512], dt.float32r, tag="lin2w")
            nc.sync.dma_start(lin2w_s[:].rearrange("p (a g) -> p a g", a=8),
                              D['lin2wT'][:].rearrange("a p g -> p a g"))
            wih0_s = cpool.tile([128, 24 * G4], dt.bfloat16, tag="wih0")
            nc.sync.dma_start(wih0_s[:].rearrange("p (a c g) -> p a c g", a=2, c=12),
                              D['wih0T'][:].rearrange("a c p g -> p a c g"))
            b0_s = ld_stack('b0row', 1, 2, dt.bfloat16, G4)
            whh0_s = ld_stack('whh0T', 100, 2, dt.bfloat16, G4)
            wih1_s = ld_stack('wih1T', 100, 2, dt.bfloat16, G4)
            whh1_s = ld_stack('whh1T', 101, 2, dt.bfloat16, G4)
            fc1w_s = ld_stack('fc1wT', 100, 2, dt.bfloat16, 512)
            fc1b_s = ld('fc1bcol', [128, 4], dt.float32)
            fc2w_s = ld_stack('fc2wT', 128, 4, dt.bfloat16, OUT)
            fc2b_s = ld('fc2brow', [1, OUT], dt.bfloat16)

            # ---- cross-phase activation tiles ----
            dataT_s = apool.tile([128, 12 * FPC], dt.bfloat16, tag="dataT")
            nc.sync.dma_start(dataT_s[:].rearrange("p (c f) -> p c f", c=12),
                              D['dataT'][:].rearrange("c p f -> p c f"))
            h1T_s = apool.tile([128, 8 * FPC], dt.float32r, tag="h1T")
            doutT_s = apool.tile([128, 4 * FPC], dt.bfloat16, tag="doutT")
            c3_s = apool.tile([128, FPC * 8], dt.bfloat16, tag="c3")

            # =========================== MLP ===========================
            with tc.tile_pool(name="psmlp", bufs=2, space="PSUM") as psm:
                l1v = lin1w_s[:].rearrange("p (a g) -> p a g", a=12)
                for m in range(8):
                    ph = psm.tile([128, FPC], dt.float32, tag="ph1")
                    for kc in range(12):
                        nc.tensor.matmul(ph[:], l1v[:, kc, 128 * m:128 * m + 128],
                                         dataT_s[:, FPC * kc:FPC * (kc + 1)],
                                         start=(kc == 0), stop=(kc == 11))
                    nc.scalar.activation(
                        h1T_s[:, FPC * m:FPC * (m + 1)].bitcast(dt.float32r),
                        ph[:], AF.Relu, bias=b1c_s[:, m:m + 1], scale=1.0)
                l2v = lin2w_s[:].rearrange("p (a g) -> p a g", a=8)
                for m in range(4):
                    pd = psm.tile([128, FPC], dt.float32, tag="pd")
                    for kc in range(8):
                        nc.tensor.matmul(pd[:], l2v[:, kc, 128 * m:128 * m + 128],
                                         h1T_s[:, FPC * kc:FPC * (kc + 1)],
                                         start=(kc == 0), stop=(kc == 7))
                    nc.vector.tensor_scalar_add(
                        doutT_s[:, FPC * m:FPC * (m + 1)],
                        pd[:], b2c_s[:, m:m + 1])

            # ======================== conv trunk ========================
            with tc.tile_pool(name="psconv", bufs=2, space="PSUM") as psc:
                for blk in range(NBLK):
                    f0 = blk * FB
                    tb = vpool.tile([48, FB * 256], dt.bfloat16, tag="tblk")
                    nc.sync.dma_start(tb[:].rearrange("p (f x) -> p f x", f=FB),
                                      D['obsT'][:, f0:f0 + FB, :])
                    tv = tb[:].rearrange("p (f by bx) -> p f by bx", by=16, bx=16)
                    rt = vpool.tile([128, FB * 90], dt.bfloat16, tag="rtile", bufs=1)
                    for gi, (a, b) in enumerate(
                            [(i, min(i + 5, FB)) for i in range(0, FB, 5)]):
                        ncols = (b - a) * 90
                        ps1 = psc.tile([128, 512], dt.float32, tag="ps1", bufs=3)
                        for j in range(4):
                            for di, (dy, dx) in enumerate(
                                    [(0, 0), (0, 1), (1, 0), (1, 1)]):
                                mov = tv[:, a:b, j + dy:j + dy + 11:2, dx:dx + 15]
                                nc.tensor.matmul(
                                    ps1[32 * j:32 * j + 32, :ncols],
                                    w1_s[:, 32 * (2 * dy + dx):32 * (2 * dy + dx) + 32],
                                    mov, start=(di == 0), stop=(di == 3),
                                    tile_position=(0, 32 * j))
                        dst = rt[:, a * 90:b * 90]
                        if gi % 2 == 0:
                            nc.scalar.activation(dst, ps1[:, :ncols], AF.Relu,
                                                 bias=cb1_s[:], scale=1.0)
                        else:
                            nc.vector.scalar_tensor_tensor(
                                dst, ps1[:, :ncols], cb1_s[:], zeros_s[:, :ncols],
                                ALU.add, ALU.max)
                    rv = rt[:].rearrange("p (f m x) -> p f m x", m=6, x=15)
                    c2 = vpool.tile([64, FB * 36], dt.bfloat16, tag="c2", bufs=1)
                    w2v = w2_s[:].rearrange("p (a b) -> p a b", a=4)
                    for (a, b) in [(0, 12), (12, 24), (24, 32)]:
                        ncols = (b - a) * 36
                        ps2 = psc.tile([64, 512], dt.float32, tag="ps2", bufs=2)
                        for kx in range(4):
                            mov = rv[:, a:b, :, kx:kx + 11:2]
                            nc.tensor.matmul(ps2[:, :ncols], w2v[:, kx, :], mov,
                                             start=(kx == 0), stop=(kx == 3))
                        nc.scalar.activation(c2[:, a * 36:b * 36], ps2[:, :ncols],
                                             AF.Relu, bias=cb2_s[:], scale=1.0)
                    c2v = c2[:].rearrange("p (f a b) -> p f a b", a=6, b=6)
                    w3v = w3_s[:].rearrange("p (a b) -> p a b", a=9)
                    ps3 = psc.tile([128, FB * 8], dt.float32, tag="ps3", bufs=2)
                    for g in range(2):
                        for ki, (ky, kx) in enumerate(
                                [(y, x) for y in range(3) for x in range(3)]):
                            mov = c2v[:, :, 2 * g + ky:2 * g + ky + 2, kx:kx + 4]
                            nc.tensor.matmul(ps3[64 * g:64 * g + 64, :],
                                             w3v[:, 3 * ky + kx, :], mov,
                                             start=(ki == 0), stop=(ki == 8),
                                             tile_position=(0, 64 * g))
                    nc.scalar.activation(c3_s[:, f0 * 8:(f0 + FB) * 8],
                                         ps3[:], AF.Relu, bias=cb3_s[:], scale=1.0)

            # ===================== XW precompute =====================
            xwpad = []
            for l in range(2):
                xwp = apool.tile([128, 8 * G4], dt.bfloat16, tag=f"xwpad{l}")
                xwpad.append(xwp)
            with tc.tile_pool(name="psxw", bufs=2, space="PSUM") as psx_p:
                wih0v = wih0_s[:].rearrange("p (a c g) -> p a c g", a=2, c=12)
                b0v = b0_s[:].rearrange("p (a g) -> p a g", a=2)
                for lst in range(2):
                    goff = lst * 128
                    psx = psx_p.tile([128, G4], dt.float32, tag="psx")
                    for kc in range(8):
                        stat = c3_s[:, goff * 8 + kc: goff * 8 + kc + 8 * 127 + 1: 8]
                        nc.tensor.matmul(psx[:], stat, wih0v[:, lst, kc, :],
                                         start=(kc == 0), stop=False)
                    for kc in range(8, 12):
                        stat = doutT_s[:, FPC * (kc - 8) + goff:
                                       FPC * (kc - 8) + goff + 128]
                        nc.tensor.matmul(psx[:], stat, wih0v[:, lst, kc, :],
                                         start=False, stop=False)
                    nc.tensor.matmul(psx[:], ones128_s[:], b0v[:, lst, :],
                                     start=False, stop=True)
                    xwc = lpool.tile([128, G4], dt.bfloat16, tag="xwc")
                    nc.scalar.activation(xwc[:], psx[:], AF.Copy, bias=0.0, scale=1.0)
                    # scatter via DRAM roundtrip (multi-level-partition SBUF
                    # DMAs read garbage): rows (b,t)=32b+4tg+tau -> pad row
                    # 32tau+b, col 400tg+g
                    nc.sync.dma_start(xw_scr[lst], xwc[:])
                    srcv = xw_scr[lst].rearrange("(b tg tau) g -> tau b tg g",
                                                 b=4, tg=8)
                    padv = xwpad[lst][:].rearrange("(tau q) (tg g) -> tau q tg g",
                                                   tau=4, tg=8)
                    for tau in range(4):
                        nc.sync.dma_start(padv[tau, 0:4], srcv[tau])

            # ========================= LSTM =========================
            hT = []
            for i in range(2):
                hTt = lpool.tile([101, 36], dt.bfloat16, tag=f"hT{i}", bufs=1)
                hT.append(hTt)
            for i in range(2):
                nc.sync.dma_start(hT[i][:], D['hT_init'][:])
            h2t = lpool.tile([36, 100], dt.bfloat16, tag="h2t", bufs=1)
            nc.sync.dma_start(h2t[:], D['zerosbf'][:])
            sC = lpool.tile([36, 100], dt.float32, tag="sC", bufs=1)
            nc.sync.dma_start(sC[:], D['zeros128'][0:36, 0:100])
            qsave = lpool.tile([100, 4], dt.bfloat16, tag="qsave", bufs=1)

            whh0v = whh0_s[:].rearrange("p (a g) -> p a g", a=2)
            wih1v = wih1_s[:].rearrange("p (a g) -> p a g", a=2)
            whh1v = whh1_s[:].rearrange("p (a g) -> p a g", a=2)

            with tc.tile_pool(name="pslstm", bufs=2, space="PSUM") as psl:
                for s_ in range(NSTEP):
                    l0_act = s_ <= 63
                    l1_act = 1 <= s_
                    lo = 0 if s_ < 32 else 1
                    l1i = 0 if (s_ - 1) < 32 else 1
                    t0m = s_ % 32
                    hp = hT[(s_ - 1) % 2]
                    hn = hT[s_ % 2]
                    p0, p1 = (0, 4) if s_ == 0 else ((32, 36) if s_ == 64 else (0, 36))

                    G = psl.tile([36, G4], dt.float32, tag="gpsum", bufs=4)
                    if l0_act:
                        nc.tensor.matmul(G[0:4, :], hp[0:100, 0:4], whh0v[:, lo, :],
                                         start=True, stop=False, tile_position=(0, 0))
                        xs = xwpad[lo][:].rearrange("p (tg g) -> p tg g", tg=8)
                        q4 = 32 * (t0m % 4)
                        nc.tensor.matmul(G[0:4, :], i4_s[q4:q4 + 4, :],
                                         xs[q4:q4 + 4, t0m // 4, :],
                                         start=False, stop=True,
                                         tile_position=(q4, 0))
                    if l1_act:
                        nc.tensor.matmul(G[32:36, :], hp[0:100, 0:4], wih1v[:, l1i, :],
                                         start=True, stop=False, tile_position=(0, 32))
                        nc.tensor.matmul(G[32:36, :], hp[0:101, 32:36], whh1v[:, l1i, :],
                                         start=False, stop=True, tile_position=(0, 32))

                    T = lpool.tile([36, G4], dt.float32, tag="tanhT")
                    nc.scalar.activation(T[p0:p1, :], G[p0:p1, :], AF.Tanh,
                                         bias=0.0, scale=0.5)
                    u = lpool.tile([36, 100], dt.float32, tag="ut")
                    v = lpool.tile([36, 100], dt.float32, tag="vt")
                    cn = lpool.tile([36, 100], dt.float32, tag="cnt")
                    th = lpool.tile([36, 100], dt.float32, tag="tht")
                    nc.vector.scalar_tensor_tensor(u[p0:p1, :], T[p0:p1, 0:100], 1.0,
                                                   sC[p0:p1, :], ALU.add, ALU.mult)
                    nc.vector.scalar_tensor_tensor(v[p0:p1, :], T[p0:p1, 100:200], 1.0,
                                                   T[p0:p1, 300:400], ALU.add, ALU.mult)
                    nc.vector.scalar_tensor_tensor(cn[p0:p1, :], v[p0:p1, :], 0.5,
                                                   u[p0:p1, :], ALU.mult, ALU.add)
                    nc.vector.tensor_scalar_mul(sC[p0:p1, :], cn[p0:p1, :], 0.5)
                    nc.scalar.activation(th[p0:p1, :], cn[p0:p1, :], AF.Tanh,
                                         bias=0.0, scale=1.0)
                    nc.vector.scalar_tensor_tensor(
                        h2t[p0:p1, :].bitcast(dt.bfloat16), T[p0:p1, 200:300], 1.0,
                        th[p0:p1, :], ALU.add, ALU.mult)
                    pst = psl.tile([100, 36], dt.bfloat16, tag="pshT", bufs=2)
                    nc.tensor.transpose(pst[:], h2t[:], id36_s[:])
                    nc.vector.tensor_copy(hn[0:100, :], pst[:])
                    if s_ == 32:
                        nc.scalar.copy(qsave[:], hn[0:100, 32:36])

            # ========================= head =========================
            with tc.tile_pool(name="pshead", bufs=1, space="PSUM") as psh:
                hlast = hT[(NSTEP - 1) % 2]
                psf1 = psh.tile([128, 16], dt.float32, tag="psf1")
                fc1v = fc1w_s[:].rearrange("p (a g) -> p a g", a=2)
                for m in range(4):
                    nc.tensor.matmul(psf1[:, 4 * m:4 * m + 4],
                                     fc1v[:, 0, 128 * m:128 * m + 128],
                                     hlast[0:100, 32:36], start=True, stop=False)
                    nc.tensor.matmul(psf1[:, 4 * m:4 * m + 4],
                                     fc1v[:, 1, 128 * m:128 * m + 128],
                                     qsave[:], start=False, stop=True)
                z1 = lpool.tile([128, 16], dt.bfloat16, tag="z1t", bufs=1)
                for m in range(4):
                    nc.scalar.activation(z1[:, 4 * m:4 * m + 4],
                                         psf1[:, 4 * m:4 * m + 4], AF.Relu,
                                         bias=fc1b_s[:, m:m + 1], scale=1.0)
                psf2 = psh.tile([4, OUT], dt.float32, tag="psf2")
                fc2v = fc2w_s[:].rearrange("p (a g) -> p a g", a=4)
                for m in range(4):
                    nc.tensor.matmul(psf2[:], z1[:, 4 * m:4 * m + 4], fc2v[:, m, :],
                                     start=(m == 0), stop=False)
                nc.tensor.matmul(psf2[:], ones4_s[:], fc2b_s[:],
                                 start=False, stop=True)
                ot = lpool.tile([4, OUT], dt.float32, tag="outt", bufs=1)
                nc.scalar.copy(ot[:], psf2[:])
                nc.sync.dma_start(out_d[:], ot[:])

            if debug:
                nc.sync.dma_start(dbg['d_c3'][:], c3_s[:])
                nc.sync.dma_start(dbg['d_dout'][:], doutT_s[:])
                for l in range(2):
                    nc.sync.dma_start(dbg['d_xw'][l], xwpad[l][:])
                nc.sync.dma_start(dbg['d_hT'][:], hT[(NSTEP - 1) % 2][:])
                nc.sync.dma_start(dbg['d_q'][:], qsave[:])

    _fix_sync_waits(nc)
    return nc


_NC_CACHE = {}


def _run(inputs, debug=False):
    inputs = {k: np.asarray(v) for k, v in inputs.items()}
    winp = {k: (np.asarray(v, np.float32) if np.asarray(v).ndim else v)
            for k, v in inputs.items()}
    w = _prep_weights(winp)
    key = ('nc', debug)
    if key not in _NC_CACHE:
        _NC_CACHE[key] = _build_nc(debug=debug)
    nc = _NC_CACHE[key]
    in_maps = [_prep_core_inputs(inputs, w, k) for k in range(N_CORES)]
    res = run_bass_kernel_spmd(nc, in_maps, core_ids=list(range(N_CORES)))
    return res


def kernel(**inputs):
    res = _run(inputs, debug=False)
    out = np.concatenate([res.results[k]['out'] for k in range(N_CORES)], 0)
    return out.astype(np.float32)

